# revision 9
# baseline (speedup 1.0000x reference)
"""Trainium2 Bass kernel for nn_DarkManifoldV2.

Math: the expensive op is propagator[b] = |inv(w_b I - H_sym)| with
H_sym shared across the batch.  One host-side eigendecomposition
H_sym = Q diag(lam) Q^T turns each batch inverse into
    inv(w_b I - H_sym) = Q diag(1/(w_b - lam)) Q^T
so per batch the device computes two d-scaled N^3 matmuls
(real/imag parts of the diagonal), an elementwise |.|, the two small
MLPs (env/omega), the decoherence mix and the state-bank softmax.
Data-parallel over batch: 16 batches / 8 cores = 2 per core. No
collectives; host concatenates per-core outputs.

Device-side per batch b (N=1024, H=64, k/m tiles of 128, n tiles of 512):
  omega path:  geneT = transpose(gene)  (PE transpose via identity)
               o1 = silu(w1^T @ geneT) with free-dim accum -> o1sum
               w_re = (w2^T o1sum)/N + b2
  d vectors:   t = w_re - lam; r = 1/(t^2+eta^2); d_re = t*r; d_im = -eta*r
               (computed in column layout [128, 8])
  S_re[k] = d_re * G[k], S_im[k] = d_im * G[k]   (G = Q^T, row-scaled)
  P_re[m,n] = sum_k S_re[k][:,m]^T @ G[k][:,n]   (float32r matmuls)
  prop = sqrt(P_re^2 + P_im^2)
  env path -> gamma -> decohered = coh + gamma*(classical - coh)... etc.
"""

import os
import numpy as np

B, N, H, SD, MINS = 16, 1024, 64, 64, 2
ETA = 0.01
NCORES = 8
BPC = B // NCORES      # batches per core
KT = N // 128          # 8 contraction tiles
MT = N // 128          # 8 output row tiles
NT = N // 512          # 2 output col tiles

_last_results = None   # BassKernelResults of the most recent run
_cached = None         # (key, nc) compiled-program cache


def _build_program(env_b2: float, omega_b2: float):
    import concourse.bass as bass
    import concourse.bacc as bacc
    import concourse.mybir as mybir
    import concourse.tile as tile

    f32 = mybir.dt.float32
    f32r = mybir.dt.float32r
    AF = mybir.ActivationFunctionType
    ALU = mybir.AluOpType
    AX = mybir.AxisListType

    nc = bacc.Bacc("TRN2", target_bir_lowering=False, debug=False,
                   num_devices=NCORES)

    # ---- DRAM I/O ----
    gene_d = nc.dram_tensor("gene_part", [BPC, N, H], f32, kind="ExternalInput").ap()
    coh_d = nc.dram_tensor("coh_part", [BPC, N, H], f32, kind="ExternalInput").ap()
    g_d = nc.dram_tensor("g_mat", [N, N], f32r, kind="ExternalInput").ap()
    negl_d = nc.dram_tensor("negl_col", [128, KT], f32, kind="ExternalInput").ap()
    gb_d = nc.dram_tensor("gb_row", [1, N], f32, kind="ExternalInput").ap()
    snnT_d = nc.dram_tensor("snnT", [64, MINS], f32, kind="ExternalInput").ap()
    ew1_d = nc.dram_tensor("env_w1", [H, 32], f32, kind="ExternalInput").ap()
    eb1_d = nc.dram_tensor("env_b1c", [32, 1], f32, kind="ExternalInput").ap()
    ew2_d = nc.dram_tensor("env_w2", [32, 1], f32, kind="ExternalInput").ap()
    ow1_d = nc.dram_tensor("omega_w1", [H, 32], f32, kind="ExternalInput").ap()
    ob1_d = nc.dram_tensor("omega_b1c", [32, 1], f32, kind="ExternalInput").ap()
    ow2_d = nc.dram_tensor("omega_w2", [32, 1], f32, kind="ExternalInput").ap()
    id_d = nc.dram_tensor("ident", [128, 128], f32, kind="ExternalInput").ap()

    prop_d = nc.dram_tensor("prop_part", [BPC, N, N], f32, kind="ExternalOutput").ap()
    dec_d = nc.dram_tensor("dec_part", [BPC, N, H], f32, kind="ExternalOutput").ap()
    gam_d = nc.dram_tensor("gam_part", [BPC, N], f32, kind="ExternalOutput").ap()
    sp_d = nc.dram_tensor("sp_part", [BPC, MINS], f32, kind="ExternalOutput").ap()

    with tile.TileContext(nc) as tc:
        from contextlib import ExitStack
        with ExitStack() as ctx:
            persist = ctx.enter_context(tc.tile_pool(name="persist", bufs=1))
            genep = ctx.enter_context(tc.tile_pool(name="genep", bufs=2))
            workp = ctx.enter_context(tc.tile_pool(name="workp", bufs=2))
            vecp = ctx.enter_context(tc.tile_pool(name="vecp", bufs=2))
            absp = ctx.enter_context(tc.tile_pool(name="absp", bufs=2))
            dramp = ctx.enter_context(
                tc.tile_pool(name="dramp", bufs=2, space="DRAM"))
            ps_tp = ctx.enter_context(
                tc.tile_pool(name="ps_tp", bufs=2, space="PSUM"))
            ps_mlp = ctx.enter_context(
                tc.tile_pool(name="ps_mlp", bufs=1, space="PSUM"))
            ps_acc = ctx.enter_context(
                tc.tile_pool(name="ps_acc", bufs=2, space="PSUM"))

            # ---- persistent loads ----
            g_sb = persist.tile([128, KT * N], f32r)     # G k-tiles, 32KB/part
            nc.sync.dma_start(g_sb[:, :], g_d.rearrange("(k p) n -> p k n", p=128))
            s_re_sb = persist.tile([128, KT * N], f32r)
            s_im_sb = persist.tile([128, KT * N], f32r)

            id_sb = persist.tile([128, 128], f32)
            nc.sync.dma_start(id_sb[:, :], id_d[:, :])
            negl_sb = persist.tile([128, KT], f32)
            nc.sync.dma_start(negl_sb[:, :], negl_d[:, :])
            gb_sb = persist.tile([1, N], f32)
            nc.sync.dma_start(gb_sb[:, :], gb_d[:, :])
            ew1_sb = persist.tile([H, 32], f32)
            nc.sync.dma_start(ew1_sb[:, :], ew1_d[:, :])
            eb1_sb = persist.tile([32, 1], f32)
            nc.sync.dma_start(eb1_sb[:, :], eb1_d[:, :])
            ew2_sb = persist.tile([32, 1], f32)
            nc.sync.dma_start(ew2_sb[:, :], ew2_d[:, :])
            ow1_sb = persist.tile([H, 32], f32)
            nc.sync.dma_start(ow1_sb[:, :], ow1_d[:, :])
            ob1_sb = persist.tile([32, 1], f32)
            nc.sync.dma_start(ob1_sb[:, :], ob1_d[:, :])
            ow2_sb = persist.tile([32, 1], f32)
            nc.sync.dma_start(ow2_sb[:, :], ow2_d[:, :])

            ones_row = persist.tile([1, 128], f32)
            nc.vector.memset(ones_row[:, :], 1.0)
            ones_col = persist.tile([128, 1], f32)
            nc.vector.memset(ones_col[:, :], 1.0)

            sg_row = persist.tile([1, N], f32)           # sigmoid(gamma_base)
            nc.scalar.activation(sg_row[:, :], gb_sb[:, :], AF.Sigmoid)

            # ---- per-batch input DMAs (both batches up front) ----
            gene_ts, coh_ts = [], []
            for b in range(BPC):
                gene_t = genep.tile([128, KT * H], f32, tag="gene")
                nc.sync.dma_start(
                    gene_t[:, :], gene_d[b].rearrange("(a p) h -> p a h", p=128))
                coh_t = genep.tile([128, KT * H], f32, tag="coh")
                nc.sync.dma_start(
                    coh_t[:, :], coh_d[b].rearrange("(a p) h -> p a h", p=128))
                gene_ts.append(gene_t)
                coh_ts.append(coh_t)

            for b in range(BPC):
                gene_t, coh_t = gene_ts[b], coh_ts[b]

                # ---- transpose gene -> geneT [64, N] ----
                geneT = workp.tile([64, N], f32, tag="geneT")
                for a in range(KT):
                    tp_ps = ps_tp.tile([64, 128], f32, tag="tp")
                    nc.tensor.transpose(
                        tp_ps[:, :], gene_t[:, a * H:(a + 1) * H], id_sb[:, :])
                    nc.scalar.copy(geneT[:, a * 128:(a + 1) * 128], tp_ps[:, :])

                # ---- classical mean over h (column layout) ----
                sum_col = vecp.tile([128, KT], f32, tag="sum_col")
                for a in range(KT):
                    nc.vector.tensor_reduce(
                        out=sum_col[:, a:a + 1],
                        in_=gene_t[:, a * H:(a + 1) * H],
                        axis=AX.X, op=ALU.add)
                cl_col = vecp.tile([128, KT], f32, tag="cl_col")
                nc.scalar.mul(cl_col[:, :], sum_col[:, :], 1.0 / H)

                # ---- cellsum = sum_n gene  [64,1] via accumulating matmuls ----
                cell_ps = ps_tp.tile([64, 1], f32, tag="tp")
                for a in range(KT):
                    nc.tensor.matmul(
                        cell_ps[:, :], gene_t[:, a * H:(a + 1) * H],
                        ones_col[:, :], start=(a == 0), stop=(a == KT - 1))

                # ---- state bank: [snn0.c, snn1.c, c.c] via one matmul ----
                sn3 = vecp.tile([64, 3], f32, tag="sn3")
                nc.sync.dma_start(sn3[:, 0:MINS], snnT_d[:, :])
                nc.scalar.copy(sn3[:, 2:3], cell_ps[:, :])
                stats_ps = ps_tp.tile([3, 1], f32, tag="tp")
                nc.tensor.matmul(stats_ps[:, :], sn3[:, :], sn3[:, 2:3],
                                 start=True, stop=True)
                stats_sb = vecp.tile([3, 1], f32, tag="stats_sb")
                nc.scalar.copy(stats_sb[:, :], stats_ps[:, :])
                st_dr = dramp.tile([3, 1], f32, tag="st_dr")
                nc.sync.dma_start(st_dr[:, :], stats_sb[:, :])
                row3 = vecp.tile([1, 3], f32, tag="row3")
                nc.sync.dma_start(row3[:, :], st_dr[:, :].rearrange("p o -> o p"))
                r1 = vecp.tile([1, 1], f32, tag="r1")
                nc.vector.reciprocal(r1[:, :], row3[:, 2:3])
                sc1 = vecp.tile([1, 1], f32, tag="sc1")
                # sqrt(100/|c|^2) = 10/|c|  (softmax temperature 0.1)
                nc.scalar.activation(sc1[:, :], r1[:, :], AF.Sqrt, scale=100.0)
                logit = vecp.tile([1, MINS], f32, tag="logit")
                nc.vector.tensor_scalar_mul(logit[:, :], row3[:, 0:MINS], sc1[:, :])
                negm = vecp.tile([1, 1], f32, tag="negm")
                nc.vector.tensor_reduce(out=negm[:, :], in_=logit[:, :],
                                        axis=AX.X, op=ALU.max, negate=True)
                ex = vecp.tile([1, MINS], f32, tag="ex")
                nc.scalar.activation(ex[:, :], logit[:, :], AF.Exp, bias=negm[:, :])
                s1 = vecp.tile([1, 1], f32, tag="s1")
                nc.vector.tensor_reduce(out=s1[:, :], in_=ex[:, :],
                                        axis=AX.X, op=ALU.add)
                rs = vecp.tile([1, 1], f32, tag="rs")
                nc.vector.reciprocal(rs[:, :], s1[:, :])
                probs = vecp.tile([1, MINS], f32, tag="probs")
                nc.vector.tensor_scalar_mul(probs[:, :], ex[:, :], rs[:, :])
                nc.sync.dma_start(sp_d[b], probs[:, :])

                # ---- env MLP -> gamma ----
                h1e_ps = ps_mlp.tile([32, N], f32, tag="mlp")
                for j in range(NT):
                    nc.tensor.matmul(
                        h1e_ps[:, j * 512:(j + 1) * 512], ew1_sb[:, :],
                        geneT[:, j * 512:(j + 1) * 512], start=True, stop=True)
                # silu(x+b1) = (x+b1)*sigmoid(x+b1)  (CoreSim has no Silu)
                h1e_b = workp.tile([32, N], f32, tag="h1b")
                nc.vector.tensor_scalar_add(h1e_b[:, :], h1e_ps[:, :],
                                            eb1_sb[:, :])
                h1e_sg = workp.tile([32, N], f32, tag="h1sg")
                nc.scalar.activation(h1e_sg[:, :], h1e_b[:, :], AF.Sigmoid)
                h1e = workp.tile([32, N], f32, tag="h1s")
                nc.vector.tensor_mul(h1e[:, :], h1e_b[:, :], h1e_sg[:, :])
                env_ps = ps_mlp.tile([1, N], f32, tag="mlp")
                for j in range(NT):
                    nc.tensor.matmul(
                        env_ps[:, j * 512:(j + 1) * 512], ew2_sb[:, :],
                        h1e[:, j * 512:(j + 1) * 512], start=True, stop=True)
                env_row = vecp.tile([1, N], f32, tag="env_row")
                nc.scalar.activation(env_row[:, :], env_ps[:, :], AF.Sigmoid,
                                     bias=float(env_b2))
                gamma_row = vecp.tile([1, N], f32, tag="gamma_row")
                # gamma = (env + 1) * sigmoid(gamma_base)
                t_row = vecp.tile([1, N], f32, tag="t_row")
                nc.vector.tensor_scalar_add(t_row[:, :], env_row[:, :], 1.0)
                nc.vector.tensor_mul(gamma_row[:, :], t_row[:, :], sg_row[:, :])
                nc.sync.dma_start(gam_d[b], gamma_row[:, :])
                # bounce through DRAM to get gamma in column layout
                g_dr = dramp.tile([1, N], f32, tag="g_dr")
                nc.sync.dma_start(g_dr[:, :], gamma_row[:, :])
                ge_col = vecp.tile([128, KT], f32, tag="ge_col")
                nc.sync.dma_start(
                    ge_col[:, :], g_dr[:, :].rearrange("o (a p) -> (o p) a", p=128))

                # ---- decohered = coh - ge*(coh - classical) ----
                dec_t = workp.tile([128, KT * H], f32, tag="dec")
                for a in range(KT):
                    u = vecp.tile([128, H], f32, tag="u")
                    nc.vector.tensor_scalar(
                        u[:, :], coh_t[:, a * H:(a + 1) * H],
                        cl_col[:, a:a + 1], ge_col[:, a:a + 1],
                        op0=ALU.subtract, op1=ALU.mult)
                    nc.vector.tensor_sub(
                        dec_t[:, a * H:(a + 1) * H],
                        coh_t[:, a * H:(a + 1) * H], u[:, :])
                nc.sync.dma_start(
                    dec_d[b].rearrange("(a p) h -> p a h", p=128), dec_t[:, :])

                # ---- omega MLP -> w_re ----
                o1_ps = ps_mlp.tile([32, N], f32, tag="mlp")
                for j in range(NT):
                    nc.tensor.matmul(
                        o1_ps[:, j * 512:(j + 1) * 512], ow1_sb[:, :],
                        geneT[:, j * 512:(j + 1) * 512], start=True, stop=True)
                o1_b = workp.tile([32, N], f32, tag="h1b")
                nc.vector.tensor_scalar_add(o1_b[:, :], o1_ps[:, :],
                                            ob1_sb[:, :])
                o1_sg = workp.tile([32, N], f32, tag="h1sg")
                nc.scalar.activation(o1_sg[:, :], o1_b[:, :], AF.Sigmoid)
                o1s = workp.tile([32, N], f32, tag="h1s")
                nc.vector.tensor_mul(o1s[:, :], o1_b[:, :], o1_sg[:, :])
                o1sum = vecp.tile([32, 1], f32, tag="o1sum")
                nc.vector.tensor_reduce(out=o1sum[:, :], in_=o1s[:, :],
                                        axis=AX.X, op=ALU.add)
                wsum_ps = ps_tp.tile([1, 1], f32, tag="tp")
                nc.tensor.matmul(wsum_ps[:, :], ow2_sb[:, :], o1sum[:, :],
                                 start=True, stop=True)
                scw = vecp.tile([1, 1], f32, tag="scw")
                nc.scalar.activation(scw[:, :], wsum_ps[:, :], AF.Identity,
                                     scale=1.0 / N, bias=float(omega_b2))
                # broadcast w_re to all 128 partitions via ones matmul
                scwb_ps = ps_tp.tile([128, 1], f32, tag="tp")
                nc.tensor.matmul(scwb_ps[:, :], ones_row[:, :], scw[:, :],
                                 start=True, stop=True)
                scwb = vecp.tile([128, 1], f32, tag="scwb")
                nc.scalar.copy(scwb[:, :], scwb_ps[:, :])

                # ---- d vectors in column layout [128, KT] ----
                t_col = vecp.tile([128, KT], f32, tag="t_col")
                nc.vector.tensor_scalar_add(t_col[:, :], negl_sb[:, :], scwb[:, :])
                sq_col = vecp.tile([128, KT], f32, tag="sq_col")
                nc.scalar.square(sq_col[:, :], t_col[:, :])
                den_col = vecp.tile([128, KT], f32, tag="den_col")
                nc.vector.tensor_scalar_add(den_col[:, :], sq_col[:, :], ETA * ETA)
                r_col = vecp.tile([128, KT], f32, tag="r_col")
                nc.vector.reciprocal(r_col[:, :], den_col[:, :])
                dre_col = vecp.tile([128, KT], f32, tag="dre_col")
                nc.vector.tensor_mul(dre_col[:, :], t_col[:, :], r_col[:, :])
                dim_col = vecp.tile([128, KT], f32, tag="dim_col")
                nc.scalar.mul(dim_col[:, :], r_col[:, :], -ETA)

                # ---- S = d * G (row-scaled G), split across ACT and DVE ----
                for k in range(KT):
                    nc.vector.tensor_scalar_mul(
                        s_re_sb[:, k * N:(k + 1) * N], g_sb[:, k * N:(k + 1) * N],
                        dre_col[:, k:k + 1])
                    nc.vector.tensor_scalar_mul(
                        s_im_sb[:, k * N:(k + 1) * N], g_sb[:, k * N:(k + 1) * N],
                        dim_col[:, k:k + 1])

                # ---- big matmuls + abs ----
                for m in range(MT):
                    for n in range(NT):
                        acc_re = ps_acc.tile([128, 512], f32, tag="acc_re")
                        acc_im = ps_acc.tile([128, 512], f32, tag="acc_im")
                        for k in range(KT):
                            nc.tensor.matmul(
                                acc_re[:, :],
                                s_re_sb[:, k * N + m * 128:k * N + (m + 1) * 128],
                                g_sb[:, k * N + n * 512:k * N + (n + 1) * 512],
                                start=(k == 0), stop=(k == KT - 1))
                        for k in range(KT):
                            nc.tensor.matmul(
                                acc_im[:, :],
                                s_im_sb[:, k * N + m * 128:k * N + (m + 1) * 128],
                                g_sb[:, k * N + n * 512:k * N + (n + 1) * 512],
                                start=(k == 0), stop=(k == KT - 1))
                        t1 = absp.tile([128, 512], f32, tag="t1")
                        nc.scalar.square(t1[:, :], acc_re[:, :])
                        t2 = absp.tile([128, 512], f32, tag="t2")
                        nc.scalar.square(t2[:, :], acc_im[:, :])
                        t3 = absp.tile([128, 512], f32, tag="t3")
                        nc.vector.tensor_add(t3[:, :], t1[:, :], t2[:, :])
                        ot = absp.tile([128, 512], f32, tag="ot")
                        nc.scalar.sqrt(ot[:, :], t3[:, :])
                        nc.sync.dma_start(
                            prop_d[b, m * 128:(m + 1) * 128,
                                   n * 512:(n + 1) * 512], ot[:, :])

    nc.compile()
    return nc


def _host_prep(inputs):
    gene = np.ascontiguousarray(np.asarray(inputs["gene_state"], dtype=np.float32))
    coh = np.ascontiguousarray(np.asarray(inputs["coherent_state"], dtype=np.float32))
    se = np.asarray(inputs["state_embeddings"], dtype=np.float32)
    gb = np.asarray(inputs["gamma_base"], dtype=np.float32)
    ew1 = np.ascontiguousarray(np.asarray(inputs["env_w1"], dtype=np.float32))
    eb1 = np.asarray(inputs["env_b1"], dtype=np.float32)
    ew2 = np.ascontiguousarray(np.asarray(inputs["env_w2"], dtype=np.float32))
    eb2 = np.asarray(inputs["env_b2"], dtype=np.float32)
    ow1 = np.ascontiguousarray(np.asarray(inputs["omega_w1"], dtype=np.float32))
    ob1 = np.asarray(inputs["omega_b1"], dtype=np.float32)
    ow2 = np.ascontiguousarray(np.asarray(inputs["omega_w2"], dtype=np.float32))
    ob2 = np.asarray(inputs["omega_b2"], dtype=np.float32)
    Hm = np.asarray(inputs["Hmat"], dtype=np.float32)

    Hs = 0.5 * (Hm.astype(np.float64) + Hm.astype(np.float64).T)
    lam, Q = np.linalg.eigh(Hs)
    G = np.ascontiguousarray(Q.T).astype(np.float32)        # rows = eigenindex
    negl_col = np.ascontiguousarray(
        (-lam).astype(np.float32).reshape(KT, 128).T)       # [p,k] = -lam[k*128+p]

    sn = se[:MINS].astype(np.float64)
    nrm = np.maximum(np.linalg.norm(sn, axis=1, keepdims=True), 1e-12)
    snnT = np.ascontiguousarray((sn / nrm).T.astype(np.float32))  # [64, 2]

    shared = {
        "g_mat": G,
        "negl_col": negl_col,
        "gb_row": np.ascontiguousarray(gb.reshape(1, N)),
        "snnT": snnT,
        "env_w1": ew1,
        "env_b1c": np.ascontiguousarray(eb1.reshape(32, 1)),
        "env_w2": ew2,
        "omega_w1": ow1,
        "omega_b1c": np.ascontiguousarray(ob1.reshape(32, 1)),
        "omega_w2": ow2,
        "ident": np.eye(128, dtype=np.float32),
    }
    per_core = []
    for c in range(NCORES):
        m = dict(shared)
        m["gene_part"] = np.ascontiguousarray(gene[c * BPC:(c + 1) * BPC])
        m["coh_part"] = np.ascontiguousarray(coh[c * BPC:(c + 1) * BPC])
        per_core.append(m)
    return per_core, float(eb2.reshape(-1)[0]), float(ob2.reshape(-1)[0])


def _get_program(env_b2, omega_b2):
    global _cached
    key = (env_b2, omega_b2)
    if _cached is not None and _cached[0] == key:
        return _cached[1]
    nc = _build_program(env_b2, omega_b2)
    _cached = (key, nc)
    return nc


def _ensure_axon_hooks():
    """bass_utils imports antenv.axon_hooks when tracing; this image's
    antenv package lacks it. Install a functional shim (real ctypes NTFF
    hook when available, else None -> bass_utils skips tracing)."""
    import sys
    import types
    try:
        import antenv.axon_hooks  # noqa: F401
        return
    except ImportError:
        pass
    mod = types.ModuleType("antenv.axon_hooks")
    _hook = [None]
    try:
        from trn_agent_boot.trn_boot import _ntff_profile_via_ctypes
        so = "/opt/axon/libaxon_pjrt.so"
        if os.path.exists(so):
            _hook[0] = _ntff_profile_via_ctypes(so)
    except Exception:
        pass
    mod.get_axon_ntff_profile_hook = lambda: _hook[0]
    mod.set_axon_ntff_profile_hook = lambda h: _hook.__setitem__(0, h)
    sys.modules["antenv.axon_hooks"] = mod
    import antenv
    antenv.axon_hooks = mod


def kernel(**inputs):
    global _last_results
    _ensure_axon_hooks()
    from concourse import bass_utils

    in_maps, env_b2, omega_b2 = _host_prep(inputs)
    nc = _get_program(env_b2, omega_b2)

    trace = bool(os.environ.get("BASS_TRACE"))
    res = bass_utils.run_bass_kernel_spmd(
        nc, in_maps, core_ids=list(range(NCORES)), trace=trace)
    _last_results = res

    sp = np.concatenate([res.results[c]["sp_part"] for c in range(NCORES)], 0)
    dec = np.concatenate([res.results[c]["dec_part"] for c in range(NCORES)], 0)
    gam = np.concatenate([res.results[c]["gam_part"] for c in range(NCORES)], 0)
    prop = np.concatenate([res.results[c]["prop_part"] for c in range(NCORES)], 0)
    return (sp.astype(np.float32), dec.astype(np.float32),
            gam.astype(np.float32), prop.astype(np.float32))
